# revision 52
# baseline (speedup 1.0000x reference)
"""Conformer block kernel for 8 Trainium2 NeuronCores.

Sharding: pure data-parallel over batch (B=8 -> 1 batch element per core,
zero collectives). All weights are replicated; BatchNorm affines and scalar
multipliers are folded into the adjacent pointwise-conv weights on the host.
The dense depthwise-conv (85% of FLOPs) runs in fp8 e4m3 with DoubleRow
perf mode (2 MACs/PE/cycle); all other matmuls run as float32r.
"""
import sys

sys.path.insert(0, '/opt/trn_rl_repo')

import numpy as np
import ml_dtypes
import concourse.bass as bass
import concourse.tile as tile
from concourse import bacc, mybir
from concourse.bass_utils import run_bass_kernel_spmd

F32 = mybir.dt.float32
F32R = mybir.dt.float32r
F8 = mybir.dt.float8e4
NP_F8 = ml_dtypes.float8_e4m3
AF = mybir.ActivationFunctionType
ALU = mybir.AluOpType
DR = mybir.MatmulPerfMode.DoubleRow

B, DIM, S = 8, 512, 1024
H, DH = 8, 64
FF_INNER = 1024
CONV_INNER = 1024
K = 31
PAD = (K - 1) // 2
N_CORES = 8

CT = DIM // 128          # 4  channel tiles of the 512-dim stream
UT = CONV_INNER // 128   # 8  tiles of 1024-wide inner dims
SC = S // 512            # 2  free-dim chunks of 512
CP = UT // 2             # 4  channel-tile PAIRS for fp8 DoubleRow contraction
SW = 512.0               # fp8 scale on quantized weights
SU = 32.0                # fp8 scale on GLU activations (folded into pw1 a-half)
SX = 16.0                # fp8 scale on residual-stream moving operands
UROW = S + 2 * PAD + 2   # 1056: u8 subtile row stride, padded to 16B multiple


def _xq8(xb):
    """Host-quantized fp8 pair-interleaved input [2, 128, 2, S]."""
    a = np.clip(xb * SX, -240, 240).astype(NP_F8)
    return np.ascontiguousarray(a.reshape(2, 2, 128, S).transpose(0, 2, 1, 3))


def _q8w(wT, scale=SW):
    """fp8 DoubleRow pair strips: [in, out] -> [in//256, 128, 2, out]."""
    n_in, n_out = wT.shape
    a = np.clip(wT * scale, -240, 240).astype(NP_F8)
    a = a.reshape(n_in // 256, 2, 128, n_out).transpose(0, 2, 1, 3)
    return np.ascontiguousarray(a)


def _host_prep(i):
    """Fold affines/scalars into weights; pre-transpose for lhsT layout.

    The residual stream on device omits the mm-output biases of ff1-mm2,
    out_proj and ff2-mm2 (their stt slot is used for the fp8 PSUM rescale
    instead); those biases are a per-channel shift absorbed into every
    downstream BatchNorm fold and the final affine.
    """
    f = np.float32
    w = {}

    def fold(wmat, g, b, bout):
        # y = wmat @ (g*x + b) + bout  ->  W' = wmat * g[None, :],
        # b' = wmat @ b + bout ; return transposed W' [in, out]
        wp = (wmat * g[None, :]).astype(f)
        bp = (wmat @ b + bout).astype(f)
        return np.ascontiguousarray(wp.T), bp

    wT, w['b_ff1_1'] = fold(i['ff1_w1'], i['ff1_g'], i['ff1_b'], i['ff1_b1'])
    w['w_ff1_1'] = _q8w(wT)
    w['w_ff1_2'] = _q8w((0.5 * i['ff1_w2']).T.astype(f))
    shift = (0.5 * i['ff1_b2']).astype(f)          # bias the stream is missing

    b_eff = (i['attn_b'] + i['attn_g'] * shift).astype(f)
    wT, w['b_q'] = fold(i['wq'], i['attn_g'], b_eff, i['bq'])
    w['w_q'] = _q8w(wT)
    wT, w['b_k'] = fold(i['wk'], i['attn_g'], b_eff, i['bk'])
    w['w_k'] = _q8w(wT)
    wT, w['b_v'] = fold(i['wv'], i['attn_g'], b_eff, i['bv'])
    w['w_v'] = _q8w(wT)
    w['w_o'] = _q8w(i['wo'].T.astype(f))
    shift = shift + i['bo']

    b_eff = (i['conv_b'] + i['conv_g'] * shift).astype(f)
    w['w_pw1'], w['b_pw1'] = fold(i['pw1_w'], i['conv_g'], b_eff, i['pw1_b'])
    # scale the GLU 'a' half by SU so the fp8 cast of the GLU output uses
    # the e4m3 range well; undone in the conv-out activation scale
    w['w_pw1'][:, :CONV_INNER] *= SU
    w['b_pw1'][:CONV_INNER] *= SU
    # dconv: fold cbn_g into weights; bias = cbn_g*dconv_b + cbn_b
    dw = (i['dconv_w'][:, :, 0, :] * i['cbn_g'][:, None, None]).astype(f)  # [o,c,k]
    dq = np.clip(dw * SW, -240, 240).astype(NP_F8)
    # DoubleRow lhsT layout [ot, ctp, ci(128), k, j(2), co(128)]: each
    # (ot, ctp) block is a [128, K*2, 128] strip; slice [:, 2k:2k+2, :]
    # is the [Ki=128, Ko=2, M=128] stationary operand for tap k
    dqt = dq.reshape(UT, 128, CP, 2, 128, K)       # [ot, co, ctp, j, ci, k]
    dqt = dqt.transpose(0, 2, 4, 5, 3, 1)          # [ot, ctp, ci, k, j, co]
    w['w_dc'] = np.ascontiguousarray(dqt).reshape(UT, CP, 128, K * 2, 128)
    w['b_dc'] = (i['cbn_g'] * i['dconv_b'] + i['cbn_b']).astype(f)
    w['w_pw2'] = np.ascontiguousarray(i['pw2_w'].T.astype(f))
    w['b_pw2'] = i['pw2_b'].astype(f)  # kept inline: pw2 residual stt has add

    b_eff = (i['ff2_b'] + i['ff2_g'] * shift).astype(f)
    wT, w['b_ff2_1'] = fold(i['ff2_w1'], i['ff2_g'], b_eff, i['ff2_b1'])
    w['w_ff2_1'] = _q8w(wT)
    w['w_ff2_2'] = _q8w((0.5 * i['ff2_w2']).T.astype(f))
    shift = shift + 0.5 * i['ff2_b2']

    w['fin_g'] = i['fin_g'].astype(f)
    w['fin_b'] = (i['fin_b'] + i['fin_g'] * shift).astype(f)
    return w


def _bias_tile(nc, sb, dram_vec, n):
    """Load a [n*128] DRAM vector as a [128, n] SBUF tile (col t = tile t)."""
    t = sb.tile([128, n], F32, tag=f'bias_{dram_vec.name}', name=f'b_{dram_vec.name}')
    nc.sync.dma_start(t[:], dram_vec.ap().rearrange('(t p) -> p t', p=128))
    return t


def _bcast_tile3(nc, sb, dram_vec, h, n, tag):
    """Broadcast a [h*n] DRAM vector across 128 partitions -> [128, h, n]."""
    t = sb.tile([128, h, n], F32R, tag=tag, name=tag)
    v = dram_vec.ap()
    nc.sync.dma_start(
        t[:], bass.AP(tensor=v.tensor, offset=0, ap=[[0, 128], [n, h], [1, n]]))
    return t


def build_program():
    nc = bacc.Bacc('TRN2', target_bir_lowering=False, debug=False)
    dt_in = {}

    def din(name, shape, dt=F32R):
        dt_in[name] = nc.dram_tensor(name, shape, dt, kind='ExternalInput')
        return dt_in[name]

    x_d = din('x', [DIM, S])
    x8_d = din('x8', [2, 128, 2, S], F8)
    w_ff1_1 = din('w_ff1_1', [2, 128, 2, FF_INNER], F8)
    b_ff1_1 = din('b_ff1_1', [FF_INNER], F32)
    w_ff1_2 = din('w_ff1_2', [4, 128, 2, DIM], F8)
    w_q = din('w_q', [2, 128, 2, DIM], F8); b_q = din('b_q', [DIM], F32)
    w_k = din('w_k', [2, 128, 2, DIM], F8); b_k = din('b_k', [DIM], F32)
    w_v = din('w_v', [2, 128, 2, DIM], F8); b_v = din('b_v', [DIM])
    w_o = din('w_o', [2, 128, 2, DIM], F8)
    w_pw1 = din('w_pw1', [DIM, 2 * CONV_INNER]); b_pw1 = din('b_pw1', [2 * CONV_INNER], F32)
    w_dc = din('w_dc', [UT, CP, 128, K * 2, 128], F8)
    b_dc = din('b_dc', [CONV_INNER], F32)
    w_pw2 = din('w_pw2', [CONV_INNER, DIM]); b_pw2 = din('b_pw2', [DIM], F32)
    w_ff2_1 = din('w_ff2_1', [2, 128, 2, FF_INNER], F8)
    b_ff2_1 = din('b_ff2_1', [FF_INNER], F32)
    w_ff2_2 = din('w_ff2_2', [4, 128, 2, DIM], F8)
    fin_g = din('fin_g', [DIM], F32); fin_b = din('fin_b', [DIM], F32)
    out_d = nc.dram_tensor('out', [DIM, S], F32, kind='ExternalOutput')

    with tile.TileContext(nc, pool_alloc_mode='queue') as tc:
        _emit(nc, tc, dt_in, out_d)
    nc.compile()
    return nc


def _emit(nc, tc, din, out_d):
    from contextlib import ExitStack
    ctx = ExitStack()
    with ctx:
        # ---- persistent pools -------------------------------------------
        resid = ctx.enter_context(tc.tile_pool(name='resid', bufs=2))
        hid = ctx.enter_context(tc.tile_pool(name='hid', bufs=1))
        btp = ctx.enter_context(tc.tile_pool(name='biases', bufs=1))

        def new_resid(i):
            return resid.tile([128, S], F32R, tag=f'r{i}', name=f'r{i}')

        def hid_tile(i):
            return hid.tile([128, S], F32R, tag=f'h{i}', name=f'h{i}')

        # fp8 pair-interleaved copies of the residual stream (DR moving
        # operands): tiles [128, 2, S], pair pt covers channels of ct=2pt+j
        xq8 = ctx.enter_context(tc.tile_pool(name='xq8', bufs=1))

        # DMA order: x8 + ff1 weights first (they gate the first matmul),
        # then the fp32 residual x (first read at ff1-mm2), then attention
        # weights (queued before FF1's dependent waits so the sync-DMA
        # stream stays ahead of the FF1->attn transition)
        x08 = []
        for pt in range(2):
            t = xq8.tile([128, 2, S], F8, tag=f'x08_{pt}', name=f'x08_{pt}')
            nc.sync.dma_start(t[:], din['x8'].ap()[pt])
            x08.append(t)
        wff1 = ctx.enter_context(tc.tile_pool(name='wff1', bufs=1))
        ff1_w1, ff1_w2 = [], []
        for pt in range(2):
            t = wff1.tile([128, 2, FF_INNER], F8, tag=f'w1_{pt}', name=f'w1_{pt}')
            nc.sync.dma_start(t[:], din['w_ff1_1'].ap()[pt])
            ff1_w1.append(t)
        for hp in range(4):
            t = wff1.tile([128, 2, DIM], F8, tag=f'w2_{hp}', name=f'w2_{hp}')
            nc.sync.dma_start(t[:], din['w_ff1_2'].ap()[hp])
            ff1_w2.append(t)
        ff1_b1 = _bias_tile(nc, btp, din['b_ff1_1'], UT)
        x_sb = []
        for i in range(CT):
            t = new_resid(i)
            nc.sync.dma_start(t[:], din['x'].ap()[i * 128:(i + 1) * 128, :])
            x_sb.append(t)

        wattn = ctx.enter_context(tc.tile_pool(name='wattn', bufs=1))
        wq_sb, wk_sb, wv_sb, wo_sb = [], [], [], []
        for nm, lst in (('w_q', wq_sb), ('w_k', wk_sb), ('w_v', wv_sb),
                        ('w_o', wo_sb)):
            for pt in range(2):
                t = wattn.tile([128, 2, DIM], F8, tag=f'{nm}_{pt}',
                               name=f'{nm}_{pt}')
                nc.sync.dma_start(t[:], din[nm].ap()[pt])
                lst.append(t)
        bv_bc = _bcast_tile3(nc, wattn, din['b_v'], H, 64, 'bv_bc')
        bq_t = _bias_tile(nc, btp, din['b_q'], CT)
        bk_t = _bias_tile(nc, btp, din['b_k'], CT)

        def quant_x(x_in, tag):
            """fp8 pair-interleaved scaled copy of 4 residual tiles."""
            x8 = []
            for pt in range(2):
                t = xq8.tile([128, 2, S], F8, tag=f'{tag}_{pt}',
                             name=f'{tag}_{pt}')
                for j in range(2):
                    nc.vector.tensor_scalar(
                        t[:, j, :], x_in[2 * pt + j][:], float(SX), 0.0,
                        op0=ALU.mult, op1=ALU.add)
                x8.append(t)
            return x8

        # ---- generic FF macro (fp8 DoubleRow both matmuls) --------------
        def ff_block(x_in, x8_sb, w1d, b1d, w2d, nm, pre=None):
            with tc.tile_pool(name=f'w{nm}', bufs=1) as wp, \
                 tc.tile_pool(name=f'h{nm}', bufs=1) as hp8, \
                 tc.tile_pool(name=f'ps{nm}', bufs=4, space='PSUM') as ps:
                if pre is not None:
                    w1_sb, w2_sb, b1_t = pre
                else:
                    w1_sb = []
                    for pt in range(2):
                        t = wp.tile([128, 2, FF_INNER], F8, tag=f'w1_{pt}')
                        nc.sync.dma_start(t[:], w1d.ap()[pt])
                        w1_sb.append(t)
                    w2_sb = []
                    for hp in range(4):
                        t = wp.tile([128, 2, DIM], F8, tag=f'w2_{hp}')
                        nc.sync.dma_start(t[:], w2d.ap()[hp])
                        w2_sb.append(t)
                    b1_t = _bias_tile(nc, btp, b1d, UT)

                h8 = [hp8.tile([128, 2, S], F8, tag=f'h8_{hp}', name=f'h8_{hp}')
                      for hp in range(4)]
                for ot in range(UT):
                    hpo, jo = ot // 2, ot % 2
                    for sc in range(SC):
                        p = ps.tile([128, 512], F32, tag='pp')
                        for pt in range(2):
                            nc.tensor.matmul(
                                p[:], w1_sb[pt][:, :, ot * 128:(ot + 1) * 128],
                                x8_sb[pt][:, :, sc * 512:(sc + 1) * 512],
                                start=(pt == 0), stop=(pt == 1), perf_mode=DR)
                        nc.scalar.activation(
                            h8[hpo][:, jo, sc * 512:(sc + 1) * 512], p[:],
                            AF.Silu, bias=b1_t[:, ot:ot + 1],
                            scale=float(1.0 / (SW * SX)))
                x_out = []
                for ot in range(CT):
                    t = new_resid(ot)
                    for sc in range(SC):
                        p = ps.tile([128, 512], F32, tag='pp')
                        for hp in range(4):
                            nc.tensor.matmul(
                                p[:], w2_sb[hp][:, :, ot * 128:(ot + 1) * 128],
                                h8[hp][:, :, sc * 512:(sc + 1) * 512],
                                start=(hp == 0), stop=(hp == 3), perf_mode=DR)
                        nc.vector.scalar_tensor_tensor(
                            t[:, sc * 512:(sc + 1) * 512], p[:],
                            float(1.0 / SW),
                            x_in[ot][:, sc * 512:(sc + 1) * 512],
                            op0=ALU.mult, op1=ALU.add)
                    x_out.append(t)
                return x_out

        # ================= FF1 =================
        x1_sb = ff_block(x_sb, x08, din['w_ff1_1'], din['b_ff1_1'],
                         din['w_ff1_2'], 'ff1', pre=(ff1_w1, ff1_w2, ff1_b1))
        x18 = quant_x(x1_sb, 'x18')

        # ================= Attention =================
        with tc.tile_pool(name='attn_sb', bufs=1) as asb, \
             tc.tile_pool(name='attn_e', bufs=6) as epool, \
             tc.tile_pool(name='attn_misc', bufs=1) as misc:
            # Q, K projections (own tiles: avoids WAR stall on ff1's h tiles)
            q_sb = [asb.tile([128, S], F32R, tag=f'q{i}', name=f'q{i}')
                    for i in range(CT)]
            k_sb = [asb.tile([128, S], F32R, tag=f'k{i}', name=f'k{i}')
                    for i in range(CT)]
            # V transposed per t-block PAIR in fp8 (DoubleRow AV contraction),
            # with a ones block per head so the AV matmul also produces the
            # softmax denominator in rows 64:128 of its PSUM output:
            # vt2[tp][:, j, h, 0:64] = v_h(t-block 2tp+j), [..., 64:128] = 1.0
            vt2 = [asb.tile([128, 2, H, 128], F8, tag=f'vt{i}', name=f'vt{i}')
                   for i in range(UT // 2)]
            with tc.tile_pool(name='ps_proj', bufs=4, space='PSUM') as ppp:
                for dst, w_sb, b_t in ((q_sb, wq_sb, bq_t), (k_sb, wk_sb, bk_t)):
                    for ot in range(CT):
                        for sc in range(SC):
                            p = ppp.tile([128, 512], F32, tag='pp')
                            for pt in range(2):
                                nc.tensor.matmul(
                                    p[:], w_sb[pt][:, :, ot * 128:(ot + 1) * 128],
                                    x18[pt][:, :, sc * 512:(sc + 1) * 512],
                                    start=(pt == 0), stop=(pt == 1),
                                    perf_mode=DR)
                            nc.scalar.activation(
                                dst[ot][:, sc * 512:(sc + 1) * 512], p[:],
                                AF.Identity, bias=b_t[:, ot:ot + 1],
                                scale=float(1.0 / (SW * SX)))
                for tt in range(UT):
                    tp, j = tt // 2, tt % 2
                    nc.vector.tensor_scalar(
                        vt2[tp][:, j, :, 64:128], bv_bc[:], 0.0, 1.0,
                        op0=ALU.mult, op1=ALU.add)
                    p = ppp.tile([128, H, 64], F32, tag='pp')
                    for pt in range(2):
                        nc.tensor.matmul(
                            p[:], x18[pt][:, :, tt * 128:(tt + 1) * 128],
                            wv_sb[pt][:], start=(pt == 0), stop=(pt == 1),
                            perf_mode=DR)
                    nc.vector.scalar_tensor_tensor(
                        vt2[tp][:, j, :, 0:64], p[:], float(1.0 / (SW * SX)),
                        bv_bc[:], op0=ALU.mult, op1=ALU.add)

            # per-head attention: two heads interleaved (keeps PE duty high so
            # HAM stays warm) and software-pipelined one tt back so the exp
            # stream on the scalar engine (the bottleneck) never stalls
            # normalized attention output in fp8 pair-interleave for the DR
            # out-projection: o8[pt][ch 0:128, j, s] = o[ct=2pt+j]
            o8 = [asb.tile([128, 2, S], F8, tag=f'o8_{i}', name=f'o8_{i}')
                  for i in range(2)]
            with tc.tile_pool(name='ps_sc', bufs=2, space='PSUM') as scp, \
                 tc.tile_pool(name='ps_av', bufs=2, space='PSUM') as avp:
                for slab in range(H // 2):
                    q_h = [q_sb[slab][po:po + 64, :] for po in (0, 64)]
                    k_h = [k_sb[slab][po:po + 64, :] for po in (0, 64)]
                    p_av = [avp.tile([128, 1024], F32, tag='av', name=f'av{hp}')
                            for hp in range(2)]

                    def av_mms(hp, e2t, tp):
                        for sc in range(SC):
                            nc.tensor.matmul(
                                p_av[hp][:, sc * 512:(sc + 1) * 512],
                                vt2[tp][:, :, slab * 2 + hp, :],
                                e2t[:, :, sc * 512:(sc + 1) * 512],
                                start=(tp == 0), stop=(tp == UT // 2 - 1),
                                perf_mode=DR)

                    # AV lags the exp stream by TWO tp-steps so the PE FIFO
                    # never places AV matmuls between a step's score matmuls
                    # and the exps that consume them: ACT runs continuously
                    epend = []
                    for tp in range(UT // 2):
                        e_cur = [epool.tile([128, 2, 1024], F8, tag='e',
                                            name=f'e{hp}') for hp in range(2)]
                        for j in range(2):
                            tt = 2 * tp + j
                            p_scs = [scp.tile([128, 1024], F32, tag='sc',
                                              name=f'sc{hp}') for hp in range(2)]
                            for sc in range(SC):
                                for hp in range(2):
                                    nc.tensor.matmul(
                                        p_scs[hp][:, sc * 512:(sc + 1) * 512],
                                        k_h[hp][:, tt * 128:(tt + 1) * 128],
                                        q_h[hp][:, sc * 512:(sc + 1) * 512],
                                        start=True, stop=True,
                                        tile_position=(64 * hp, 0))
                            for hp in range(2):
                                nc.scalar.activation(
                                    e_cur[hp][:, j, :], p_scs[hp][:], AF.Exp,
                                    scale=float(DH) ** -0.5)
                        epend.append(e_cur)
                        if len(epend) > 2:
                            e_old = epend.pop(0)
                            for hp in range(2):
                                av_mms(hp, e_old[hp], tp - 2)
                    def post(hp):
                        po = 64 * hp
                        sumc = misc.tile([64, 1024], F32, tag='sumc',
                                         name='sumc')
                        nc.vector.tensor_copy(sumc[:], p_av[hp][64:128, :])
                        rec = misc.tile([64, 1024], F32, tag='rec', name='rec')
                        nc.vector.reciprocal_approx_fast(rec[:], sumc[:])
                        nc.vector.tensor_mul(
                            o8[slab // 2][po:po + 64, slab % 2, :],
                            p_av[hp][0:64, :], rec[:])

                    for lag, e_old in enumerate(epend):
                        last = lag == len(epend) - 1
                        for hp in range(2):
                            av_mms(hp, e_old[hp], UT // 2 - 2 + lag)
                            if last:
                                post(hp)

            # out projection + residual (fp8 DR; bias absorbed downstream)
            x2_sb = []
            with tc.tile_pool(name='ps_oproj', bufs=4, space='PSUM') as ppo:
                for ot in range(CT):
                    t = new_resid(ot)
                    for sc in range(SC):
                        p = ppo.tile([128, 512], F32, tag='pp')
                        for pt in range(2):
                            nc.tensor.matmul(
                                p[:], wo_sb[pt][:, :, ot * 128:(ot + 1) * 128],
                                o8[pt][:, :, sc * 512:(sc + 1) * 512],
                                start=(pt == 0), stop=(pt == 1), perf_mode=DR)
                        nc.vector.scalar_tensor_tensor(
                            t[:, sc * 512:(sc + 1) * 512], p[:],
                            float(1.0 / SW),
                            x1_sb[ot][:, sc * 512:(sc + 1) * 512],
                            op0=ALU.mult, op1=ALU.add)
                    x2_sb.append(t)

        # ================= Conv module =================
        with tc.tile_pool(name='u8p', bufs=1) as up:
            # GLU output, fp8 e4m3 scaled by SU, all 8 channel subtiles in one
            # tile so a [:, 2c:2c+2, :] slice is a DoubleRow moving operand
            u8 = up.tile([128, UT, UROW], F8, tag='u8', name='u8')
            for i in range(UT):
                nc.any.memset(u8[:, i, 0:PAD], 0)
                nc.any.memset(u8[:, i, S + PAD:UROW], 0)
            # pw1 + GLU
            with tc.tile_pool(name='wpw1', bufs=1) as wp1, \
                 tc.tile_pool(name='sig', bufs=2) as sigp, \
                 tc.tile_pool(name='ps_pw1', bufs=4, space='PSUM') as ps1:
                pw1_sb = []
                for i in range(CT):
                    t = wp1.tile([128, 2 * CONV_INNER], F32R, tag=f'pw1_{i}')
                    nc.sync.dma_start(t[:], din['w_pw1'].ap()[i * 128:(i + 1) * 128, :])
                    pw1_sb.append(t)
                bpw1_t = _bias_tile(nc, btp, din['b_pw1'], 2 * UT)
                for ut in range(UT):
                    for sc in range(SC):
                        p_a = ps1.tile([128, 512], F32, tag='pp')
                        p_g = ps1.tile([128, 512], F32, tag='pp')
                        for ct in range(CT):
                            nc.tensor.matmul(
                                p_a[:], pw1_sb[ct][:, ut * 128:(ut + 1) * 128],
                                x2_sb[ct][:, sc * 512:(sc + 1) * 512],
                                start=(ct == 0), stop=(ct == CT - 1))
                        for ct in range(CT):
                            nc.tensor.matmul(
                                p_g[:], pw1_sb[ct][:, CONV_INNER + ut * 128:CONV_INNER + (ut + 1) * 128],
                                x2_sb[ct][:, sc * 512:(sc + 1) * 512],
                                start=(ct == 0), stop=(ct == CT - 1))
                        sig = sigp.tile([128, 512], F32, tag='sig')
                        nc.scalar.activation(sig[:], p_g[:], AF.Sigmoid,
                                             bias=bpw1_t[:, UT + ut:UT + ut + 1])
                        nc.vector.scalar_tensor_tensor(
                            u8[:, ut, PAD + sc * 512:PAD + (sc + 1) * 512],
                            p_a[:], bpw1_t[:, ut:ut + 1], sig[:],
                            op0=ALU.add, op1=ALU.mult)

            # depth conv (dense conv1d over seq, K=31) + silu, fp8 DoubleRow
            h_sb = [hid_tile(i) for i in range(UT)]
            with tc.tile_pool(name='wdc', bufs=3) as wdc, \
                 tc.tile_pool(name='ps_dc', bufs=4, space='PSUM') as psd:
                bdc_t = _bias_tile(nc, btp, din['b_dc'], UT)
                for ot in range(UT):
                    ps_c = [psd.tile([128, 512], F32, tag='cv', name=f'cv{_sc}') for _sc in range(SC)]
                    for cp in range(CP):
                        wt = wdc.tile([128, K * 2, 128], F8, tag='dw')
                        nc.sync.dma_start(wt[:], din['w_dc'].ap()[ot, cp])
                        for k in range(K):
                            for sc in range(SC):
                                nc.tensor.matmul(
                                    ps_c[sc][:], wt[:, 2 * k:2 * k + 2, :],
                                    u8[:, 2 * cp:2 * cp + 2,
                                       k + sc * 512:k + sc * 512 + 512],
                                    start=(cp == 0 and k == 0),
                                    stop=(cp == CP - 1 and k == K - 1),
                                    perf_mode=DR)
                    for sc in range(SC):
                        nc.scalar.activation(
                            h_sb[ot][:, sc * 512:(sc + 1) * 512], ps_c[sc][:],
                            AF.Silu, bias=bdc_t[:, ot:ot + 1],
                            scale=float(1.0 / (SW * SU)))

        # pw2 + residual
        x3_sb = []
        with tc.tile_pool(name='wpw2', bufs=1) as wp2, \
             tc.tile_pool(name='ps_pw2', bufs=4, space='PSUM') as ps2:
            pw2_sb = []
            for i in range(UT):
                t = wp2.tile([128, DIM], F32R, tag=f'pw2_{i}')
                nc.sync.dma_start(t[:], din['w_pw2'].ap()[i * 128:(i + 1) * 128, :])
                pw2_sb.append(t)
            bpw2_t = _bias_tile(nc, btp, din['b_pw2'], CT)
            for ot in range(CT):
                t = new_resid(ot)
                for sc in range(SC):
                    p = ps2.tile([128, 512], F32, tag='pp')
                    for ct in range(UT):
                        nc.tensor.matmul(
                            p[:], pw2_sb[ct][:, ot * 128:(ot + 1) * 128],
                            h_sb[ct][:, sc * 512:(sc + 1) * 512],
                            start=(ct == 0), stop=(ct == UT - 1))
                    nc.vector.scalar_tensor_tensor(
                        t[:, sc * 512:(sc + 1) * 512], p[:],
                        bpw2_t[:, ot:ot + 1],
                        x2_sb[ot][:, sc * 512:(sc + 1) * 512],
                        op0=ALU.add, op1=ALU.add)
                x3_sb.append(t)

        # ================= FF2 =================
        x38 = quant_x(x3_sb, 'x38')
        x4_sb = ff_block(x3_sb, x38, din['w_ff2_1'], din['b_ff2_1'],
                         din['w_ff2_2'], 'ff2')

        # ================= final affine + store =================
        with tc.tile_pool(name='fin', bufs=2) as fp:
            fing_t = _bias_tile(nc, btp, din['fin_g'], CT)
            finb_t = _bias_tile(nc, btp, din['fin_b'], CT)
            for ot in range(CT):
                o_t = fp.tile([128, S], F32, tag='out')
                nc.vector.tensor_scalar(
                    o_t[:], x4_sb[ot][:], fing_t[:, ot:ot + 1],
                    finb_t[:, ot:ot + 1], op0=ALU.mult, op1=ALU.add)
                nc.sync.dma_start(out_d.ap()[ot * 128:(ot + 1) * 128, :], o_t[:])


_prog_cache = {}


def _get_program():
    if 'nc' not in _prog_cache:
        _prog_cache['nc'] = build_program()
    return _prog_cache['nc']


def kernel(**inputs):
    inputs = {k: np.asarray(v, dtype=np.float32) for k, v in inputs.items()}
    w = _host_prep(inputs)
    nc = _get_program()
    x = inputs['x'][..., 0]  # [B, DIM, S]
    in_maps = [dict(w, x=np.ascontiguousarray(x[b]), x8=_xq8(x[b]))
               for b in range(N_CORES)]
    res = run_bass_kernel_spmd(nc, in_maps, core_ids=list(range(N_CORES)))
    out = np.stack([res.results[b]['out'] for b in range(N_CORES)])
    return out[..., None].astype(np.float32)



# revision 54
# speedup vs baseline: 1.1333x; 1.1333x over previous
"""Conformer block kernel for 8 Trainium2 NeuronCores.

Sharding: pure data-parallel over batch (B=8 -> 1 batch element per core,
zero collectives). All weights are replicated; BatchNorm affines and scalar
multipliers are folded into the adjacent pointwise-conv weights on the host.
The dense depthwise-conv (85% of FLOPs) runs in fp8 e4m3 with DoubleRow
perf mode (2 MACs/PE/cycle); all other matmuls run as float32r.
"""
import sys

sys.path.insert(0, '/opt/trn_rl_repo')

import numpy as np
import ml_dtypes
import concourse.bass as bass
import concourse.tile as tile
from concourse import bacc, mybir
from concourse.bass_utils import run_bass_kernel_spmd

F32 = mybir.dt.float32
F32R = mybir.dt.float32r
F8 = mybir.dt.float8e4
NP_F8 = ml_dtypes.float8_e4m3
AF = mybir.ActivationFunctionType
ALU = mybir.AluOpType
DR = mybir.MatmulPerfMode.DoubleRow

B, DIM, S = 8, 512, 1024
H, DH = 8, 64
FF_INNER = 1024
CONV_INNER = 1024
K = 31
PAD = (K - 1) // 2
N_CORES = 8

CT = DIM // 128          # 4  channel tiles of the 512-dim stream
UT = CONV_INNER // 128   # 8  tiles of 1024-wide inner dims
SC = S // 512            # 2  free-dim chunks of 512
CP = UT // 2             # 4  channel-tile PAIRS for fp8 DoubleRow contraction
SW = 512.0               # fp8 scale on quantized weights
SU = 32.0                # fp8 scale on GLU activations (folded into pw1 a-half)
SX = 16.0                # fp8 scale on residual-stream moving operands
UROW = S + 2 * PAD + 2   # 1056: u8 subtile row stride, padded to 16B multiple


def _xq8(xb):
    """Host-quantized fp8 pair-interleaved input [2, 128, 2, S]."""
    a = np.clip(xb * SX, -240, 240).astype(NP_F8)
    return np.ascontiguousarray(a.reshape(2, 2, 128, S).transpose(0, 2, 1, 3))


def _q8w(wT, scale=SW):
    """fp8 DoubleRow pair strips: [in, out] -> [in//256, 128, 2, out]."""
    n_in, n_out = wT.shape
    a = np.clip(wT * scale, -240, 240).astype(NP_F8)
    a = a.reshape(n_in // 256, 2, 128, n_out).transpose(0, 2, 1, 3)
    return np.ascontiguousarray(a)


def _host_prep(i):
    """Fold affines/scalars into weights; pre-transpose for lhsT layout.

    The residual stream on device omits the mm-output biases of ff1-mm2,
    out_proj and ff2-mm2 (their stt slot is used for the fp8 PSUM rescale
    instead); those biases are a per-channel shift absorbed into every
    downstream BatchNorm fold and the final affine.
    """
    f = np.float32
    w = {}

    def fold(wmat, g, b, bout):
        # y = wmat @ (g*x + b) + bout  ->  W' = wmat * g[None, :],
        # b' = wmat @ b + bout ; return transposed W' [in, out]
        wp = (wmat * g[None, :]).astype(f)
        bp = (wmat @ b + bout).astype(f)
        return np.ascontiguousarray(wp.T), bp

    wT, w['b_ff1_1'] = fold(i['ff1_w1'], i['ff1_g'], i['ff1_b'], i['ff1_b1'])
    w['w_ff1_1'] = _q8w(wT)
    w['w_ff1_2'] = _q8w((0.5 * i['ff1_w2']).T.astype(f))
    shift = (0.5 * i['ff1_b2']).astype(f)          # bias the stream is missing

    b_eff = (i['attn_b'] + i['attn_g'] * shift).astype(f)
    wT, w['b_q'] = fold(i['wq'], i['attn_g'], b_eff, i['bq'])
    w['w_q'] = _q8w(wT)
    wT, w['b_k'] = fold(i['wk'], i['attn_g'], b_eff, i['bk'])
    w['w_k'] = _q8w(wT)
    wT, w['b_v'] = fold(i['wv'], i['attn_g'], b_eff, i['bv'])
    w['w_v'] = _q8w(wT)
    w['w_o'] = _q8w(i['wo'].T.astype(f))
    shift = shift + i['bo']

    b_eff = (i['conv_b'] + i['conv_g'] * shift).astype(f)
    w['w_pw1'], w['b_pw1'] = fold(i['pw1_w'], i['conv_g'], b_eff, i['pw1_b'])
    # scale the GLU 'a' half by SU so the fp8 cast of the GLU output uses
    # the e4m3 range well; undone in the conv-out activation scale
    w['w_pw1'][:, :CONV_INNER] *= SU
    w['b_pw1'][:CONV_INNER] *= SU
    # dconv: fold cbn_g into weights; bias = cbn_g*dconv_b + cbn_b
    dw = (i['dconv_w'][:, :, 0, :] * i['cbn_g'][:, None, None]).astype(f)  # [o,c,k]
    dq = np.clip(dw * SW, -240, 240).astype(NP_F8)
    # DoubleRow lhsT layout [ot, ctp, ci(128), k, j(2), co(128)]: each
    # (ot, ctp) block is a [128, K*2, 128] strip; slice [:, 2k:2k+2, :]
    # is the [Ki=128, Ko=2, M=128] stationary operand for tap k
    dqt = dq.reshape(UT, 128, CP, 2, 128, K)       # [ot, co, ctp, j, ci, k]
    dqt = dqt.transpose(0, 2, 4, 5, 3, 1)          # [ot, ctp, ci, k, j, co]
    w['w_dc'] = np.ascontiguousarray(dqt).reshape(UT, CP, 128, K * 2, 128)
    w['b_dc'] = (i['cbn_g'] * i['dconv_b'] + i['cbn_b']).astype(f)
    w['w_pw2'] = np.ascontiguousarray(i['pw2_w'].T.astype(f))
    w['b_pw2'] = i['pw2_b'].astype(f)  # kept inline: pw2 residual stt has add

    b_eff = (i['ff2_b'] + i['ff2_g'] * shift).astype(f)
    wT, w['b_ff2_1'] = fold(i['ff2_w1'], i['ff2_g'], b_eff, i['ff2_b1'])
    w['w_ff2_1'] = _q8w(wT)
    w['w_ff2_2'] = _q8w((0.5 * i['ff2_w2']).T.astype(f))
    shift = shift + 0.5 * i['ff2_b2']

    w['fin_g'] = i['fin_g'].astype(f)
    w['fin_b'] = (i['fin_b'] + i['fin_g'] * shift).astype(f)
    return w


def _bias_tile(nc, sb, dram_vec, n):
    """Load a [n*128] DRAM vector as a [128, n] SBUF tile (col t = tile t)."""
    t = sb.tile([128, n], F32, tag=f'bias_{dram_vec.name}', name=f'b_{dram_vec.name}')
    nc.sync.dma_start(t[:], dram_vec.ap().rearrange('(t p) -> p t', p=128))
    return t


def _bcast_tile3(nc, sb, dram_vec, h, n, tag):
    """Broadcast a [h*n] DRAM vector across 128 partitions -> [128, h, n]."""
    t = sb.tile([128, h, n], F32R, tag=tag, name=tag)
    v = dram_vec.ap()
    nc.sync.dma_start(
        t[:], bass.AP(tensor=v.tensor, offset=0, ap=[[0, 128], [n, h], [1, n]]))
    return t


def build_program():
    nc = bacc.Bacc('TRN2', target_bir_lowering=False, debug=False)
    dt_in = {}

    def din(name, shape, dt=F32R):
        dt_in[name] = nc.dram_tensor(name, shape, dt, kind='ExternalInput')
        return dt_in[name]

    x_d = din('x', [DIM, S])
    x8_d = din('x8', [2, 128, 2, S], F8)
    w_ff1_1 = din('w_ff1_1', [2, 128, 2, FF_INNER], F8)
    b_ff1_1 = din('b_ff1_1', [FF_INNER], F32)
    w_ff1_2 = din('w_ff1_2', [4, 128, 2, DIM], F8)
    w_q = din('w_q', [2, 128, 2, DIM], F8); b_q = din('b_q', [DIM], F32)
    w_k = din('w_k', [2, 128, 2, DIM], F8); b_k = din('b_k', [DIM], F32)
    w_v = din('w_v', [2, 128, 2, DIM], F8); b_v = din('b_v', [DIM])
    w_o = din('w_o', [2, 128, 2, DIM], F8)
    w_pw1 = din('w_pw1', [DIM, 2 * CONV_INNER]); b_pw1 = din('b_pw1', [2 * CONV_INNER], F32)
    w_dc = din('w_dc', [UT, CP, 128, K * 2, 128], F8)
    b_dc = din('b_dc', [CONV_INNER], F32)
    w_pw2 = din('w_pw2', [CONV_INNER, DIM]); b_pw2 = din('b_pw2', [DIM], F32)
    w_ff2_1 = din('w_ff2_1', [2, 128, 2, FF_INNER], F8)
    b_ff2_1 = din('b_ff2_1', [FF_INNER], F32)
    w_ff2_2 = din('w_ff2_2', [4, 128, 2, DIM], F8)
    fin_g = din('fin_g', [DIM], F32); fin_b = din('fin_b', [DIM], F32)
    out_d = nc.dram_tensor('out', [DIM, S], F32, kind='ExternalOutput')

    with tile.TileContext(nc, pool_alloc_mode='queue') as tc:
        _emit(nc, tc, dt_in, out_d)
    nc.compile()
    return nc


def _emit(nc, tc, din, out_d):
    from contextlib import ExitStack
    ctx = ExitStack()
    with ctx:
        # ---- persistent pools -------------------------------------------
        resid = ctx.enter_context(tc.tile_pool(name='resid', bufs=2))
        hid = ctx.enter_context(tc.tile_pool(name='hid', bufs=1))
        btp = ctx.enter_context(tc.tile_pool(name='biases', bufs=1))

        def new_resid(i):
            return resid.tile([128, S], F32R, tag=f'r{i}', name=f'r{i}')

        def hid_tile(i):
            return hid.tile([128, S], F32R, tag=f'h{i}', name=f'h{i}')

        # fp8 pair-interleaved copies of the residual stream (DR moving
        # operands): tiles [128, 2, S], pair pt covers channels of ct=2pt+j
        xq8 = ctx.enter_context(tc.tile_pool(name='xq8', bufs=1))

        # load x (fp32 residual) + host-quantized x8
        x_sb = []
        for i in range(CT):
            t = new_resid(i)
            nc.sync.dma_start(t[:], din['x'].ap()[i * 128:(i + 1) * 128, :])
            x_sb.append(t)
        x08 = []
        for pt in range(2):
            t = xq8.tile([128, 2, S], F8, tag=f'x08_{pt}', name=f'x08_{pt}')
            nc.sync.dma_start(t[:], din['x8'].ap()[pt])
            x08.append(t)

        # prefetch attention weights now so the sync-DMA stream has them
        # queued before FF1's dependent waits (kills the FF1->attn PE stall)
        wattn = ctx.enter_context(tc.tile_pool(name='wattn', bufs=1))
        wq_sb, wk_sb, wv_sb, wo_sb = [], [], [], []
        for nm, lst in (('w_q', wq_sb), ('w_k', wk_sb), ('w_v', wv_sb),
                        ('w_o', wo_sb)):
            for pt in range(2):
                t = wattn.tile([128, 2, DIM], F8, tag=f'{nm}_{pt}',
                               name=f'{nm}_{pt}')
                nc.sync.dma_start(t[:], din[nm].ap()[pt])
                lst.append(t)
        bv_bc = _bcast_tile3(nc, wattn, din['b_v'], H, 64, 'bv_bc')
        bq_t = _bias_tile(nc, btp, din['b_q'], CT)
        bk_t = _bias_tile(nc, btp, din['b_k'], CT)

        def quant_x(x_in, tag):
            """fp8 pair-interleaved scaled copy of 4 residual tiles."""
            x8 = []
            for pt in range(2):
                t = xq8.tile([128, 2, S], F8, tag=f'{tag}_{pt}',
                             name=f'{tag}_{pt}')
                for j in range(2):
                    nc.vector.tensor_scalar(
                        t[:, j, :], x_in[2 * pt + j][:], float(SX), 0.0,
                        op0=ALU.mult, op1=ALU.add)
                x8.append(t)
            return x8

        # ---- generic FF macro (fp8 DoubleRow both matmuls) --------------
        def ff_block(x_in, x8_sb, w1d, b1d, w2d, nm):
            with tc.tile_pool(name=f'w{nm}', bufs=1) as wp, \
                 tc.tile_pool(name=f'h{nm}', bufs=1) as hp8, \
                 tc.tile_pool(name=f'ps{nm}', bufs=4, space='PSUM') as ps:
                w1_sb = []
                for pt in range(2):
                    t = wp.tile([128, 2, FF_INNER], F8, tag=f'w1_{pt}')
                    nc.sync.dma_start(t[:], w1d.ap()[pt])
                    w1_sb.append(t)
                w2_sb = []
                for hp in range(4):
                    t = wp.tile([128, 2, DIM], F8, tag=f'w2_{hp}')
                    nc.sync.dma_start(t[:], w2d.ap()[hp])
                    w2_sb.append(t)
                b1_t = _bias_tile(nc, btp, b1d, UT)

                h8 = [hp8.tile([128, 2, S], F8, tag=f'h8_{hp}', name=f'h8_{hp}')
                      for hp in range(4)]
                for ot in range(UT):
                    hpo, jo = ot // 2, ot % 2
                    for sc in range(SC):
                        p = ps.tile([128, 512], F32, tag='pp')
                        for pt in range(2):
                            nc.tensor.matmul(
                                p[:], w1_sb[pt][:, :, ot * 128:(ot + 1) * 128],
                                x8_sb[pt][:, :, sc * 512:(sc + 1) * 512],
                                start=(pt == 0), stop=(pt == 1), perf_mode=DR)
                        nc.scalar.activation(
                            h8[hpo][:, jo, sc * 512:(sc + 1) * 512], p[:],
                            AF.Silu, bias=b1_t[:, ot:ot + 1],
                            scale=float(1.0 / (SW * SX)))
                x_out = []
                for ot in range(CT):
                    t = new_resid(ot)
                    for sc in range(SC):
                        p = ps.tile([128, 512], F32, tag='pp')
                        for hp in range(4):
                            nc.tensor.matmul(
                                p[:], w2_sb[hp][:, :, ot * 128:(ot + 1) * 128],
                                h8[hp][:, :, sc * 512:(sc + 1) * 512],
                                start=(hp == 0), stop=(hp == 3), perf_mode=DR)
                        nc.vector.scalar_tensor_tensor(
                            t[:, sc * 512:(sc + 1) * 512], p[:],
                            float(1.0 / SW),
                            x_in[ot][:, sc * 512:(sc + 1) * 512],
                            op0=ALU.mult, op1=ALU.add)
                    x_out.append(t)
                return x_out

        # ================= FF1 =================
        x1_sb = ff_block(x_sb, x08, din['w_ff1_1'], din['b_ff1_1'],
                         din['w_ff1_2'], 'ff1')
        x18 = quant_x(x1_sb, 'x18')

        # ================= Attention =================
        with tc.tile_pool(name='attn_sb', bufs=1) as asb, \
             tc.tile_pool(name='attn_e', bufs=4) as epool, \
             tc.tile_pool(name='attn_misc', bufs=1) as misc:
            # Q, K projections (own tiles: avoids WAR stall on ff1's h tiles)
            q_sb = [asb.tile([128, S], F32R, tag=f'q{i}', name=f'q{i}')
                    for i in range(CT)]
            k_sb = [asb.tile([128, S], F32R, tag=f'k{i}', name=f'k{i}')
                    for i in range(CT)]
            # V transposed per t-block PAIR in fp8 (DoubleRow AV contraction),
            # with a ones block per head so the AV matmul also produces the
            # softmax denominator in rows 64:128 of its PSUM output:
            # vt2[tp][:, j, h, 0:64] = v_h(t-block 2tp+j), [..., 64:128] = 1.0
            vt2 = [asb.tile([128, 2, H, 128], F8, tag=f'vt{i}', name=f'vt{i}')
                   for i in range(UT // 2)]
            with tc.tile_pool(name='ps_proj', bufs=4, space='PSUM') as ppp:
                for dst, w_sb, b_t in ((q_sb, wq_sb, bq_t), (k_sb, wk_sb, bk_t)):
                    for ot in range(CT):
                        for sc in range(SC):
                            p = ppp.tile([128, 512], F32, tag='pp')
                            for pt in range(2):
                                nc.tensor.matmul(
                                    p[:], w_sb[pt][:, :, ot * 128:(ot + 1) * 128],
                                    x18[pt][:, :, sc * 512:(sc + 1) * 512],
                                    start=(pt == 0), stop=(pt == 1),
                                    perf_mode=DR)
                            nc.scalar.activation(
                                dst[ot][:, sc * 512:(sc + 1) * 512], p[:],
                                AF.Identity, bias=b_t[:, ot:ot + 1],
                                scale=float(1.0 / (SW * SX)))
                for tt in range(UT):
                    tp, j = tt // 2, tt % 2
                    nc.vector.tensor_scalar(
                        vt2[tp][:, j, :, 64:128], bv_bc[:], 0.0, 1.0,
                        op0=ALU.mult, op1=ALU.add)
                    p = ppp.tile([128, H, 64], F32, tag='pp')
                    for pt in range(2):
                        nc.tensor.matmul(
                            p[:], x18[pt][:, :, tt * 128:(tt + 1) * 128],
                            wv_sb[pt][:], start=(pt == 0), stop=(pt == 1),
                            perf_mode=DR)
                    nc.vector.scalar_tensor_tensor(
                        vt2[tp][:, j, :, 0:64], p[:], float(1.0 / (SW * SX)),
                        bv_bc[:], op0=ALU.mult, op1=ALU.add)

            # per-head attention: two heads interleaved (keeps PE duty high so
            # HAM stays warm) and software-pipelined one tt back so the exp
            # stream on the scalar engine (the bottleneck) never stalls
            # normalized attention output in fp8 pair-interleave for the DR
            # out-projection: o8[pt][ch 0:128, j, s] = o[ct=2pt+j]
            o8 = [asb.tile([128, 2, S], F8, tag=f'o8_{i}', name=f'o8_{i}')
                  for i in range(2)]
            with tc.tile_pool(name='ps_sc', bufs=2, space='PSUM') as scp, \
                 tc.tile_pool(name='ps_av', bufs=2, space='PSUM') as avp:
                for slab in range(H // 2):
                    q_h = [q_sb[slab][po:po + 64, :] for po in (0, 64)]
                    k_h = [k_sb[slab][po:po + 64, :] for po in (0, 64)]
                    p_av = [avp.tile([128, 1024], F32, tag='av', name=f'av{hp}')
                            for hp in range(2)]

                    def av_mms(hp, e2t, tp):
                        for sc in range(SC):
                            nc.tensor.matmul(
                                p_av[hp][:, sc * 512:(sc + 1) * 512],
                                vt2[tp][:, :, slab * 2 + hp, :],
                                e2t[:, :, sc * 512:(sc + 1) * 512],
                                start=(tp == 0), stop=(tp == UT // 2 - 1),
                                perf_mode=DR)

                    e_prev = [None, None]
                    for tp in range(UT // 2):
                        e_cur = [epool.tile([128, 2, 1024], F8, tag='e',
                                            name=f'e{hp}') for hp in range(2)]
                        for j in range(2):
                            tt = 2 * tp + j
                            p_scs = [scp.tile([128, 1024], F32, tag='sc',
                                              name=f'sc{hp}') for hp in range(2)]
                            # each pair issued twice: the duplicate is
                            # idempotent (same operands, full overwrite) and
                            # raises PE duty so the HAM clock-gate stays warm
                            for dup in range(2):
                                for sc in range(SC):
                                    for hp in range(2):
                                        nc.tensor.matmul(
                                            p_scs[hp][:, sc * 512:(sc + 1) * 512],
                                            k_h[hp][:, tt * 128:(tt + 1) * 128],
                                            q_h[hp][:, sc * 512:(sc + 1) * 512],
                                            start=True, stop=True,
                                            tile_position=(64 * hp, 0))
                            for hp in range(2):
                                nc.scalar.activation(
                                    e_cur[hp][:, j, :], p_scs[hp][:], AF.Exp,
                                    scale=float(DH) ** -0.5)
                        for hp in range(2):
                            if e_prev[hp] is not None:
                                av_mms(hp, e_prev[hp], tp - 1)
                        e_prev = e_cur
                    for hp in range(2):
                        av_mms(hp, e_prev[hp], UT // 2 - 1)
                    for hp in range(2):
                        po = 64 * hp
                        sumc = misc.tile([64, 1024], F32, tag='sumc')
                        nc.vector.tensor_copy(sumc[:], p_av[hp][64:128, :])
                        rec = misc.tile([64, 1024], F32, tag='rec')
                        nc.vector.reciprocal_approx_fast(rec[:], sumc[:])
                        nc.vector.tensor_mul(
                            o8[slab // 2][po:po + 64, slab % 2, :],
                            p_av[hp][0:64, :], rec[:])

            # out projection + residual (fp8 DR; bias absorbed downstream)
            x2_sb = []
            with tc.tile_pool(name='ps_oproj', bufs=4, space='PSUM') as ppo:
                for ot in range(CT):
                    t = new_resid(ot)
                    for sc in range(SC):
                        p = ppo.tile([128, 512], F32, tag='pp')
                        for pt in range(2):
                            nc.tensor.matmul(
                                p[:], wo_sb[pt][:, :, ot * 128:(ot + 1) * 128],
                                o8[pt][:, :, sc * 512:(sc + 1) * 512],
                                start=(pt == 0), stop=(pt == 1), perf_mode=DR)
                        nc.vector.scalar_tensor_tensor(
                            t[:, sc * 512:(sc + 1) * 512], p[:],
                            float(1.0 / SW),
                            x1_sb[ot][:, sc * 512:(sc + 1) * 512],
                            op0=ALU.mult, op1=ALU.add)
                    x2_sb.append(t)

        # ================= Conv module =================
        with tc.tile_pool(name='u8p', bufs=1) as up:
            # GLU output, fp8 e4m3 scaled by SU, all 8 channel subtiles in one
            # tile so a [:, 2c:2c+2, :] slice is a DoubleRow moving operand
            u8 = up.tile([128, UT, UROW], F8, tag='u8', name='u8')
            for i in range(UT):
                nc.any.memset(u8[:, i, 0:PAD], 0)
                nc.any.memset(u8[:, i, S + PAD:UROW], 0)
            # pw1 + GLU
            with tc.tile_pool(name='wpw1', bufs=1) as wp1, \
                 tc.tile_pool(name='sig', bufs=2) as sigp, \
                 tc.tile_pool(name='ps_pw1', bufs=4, space='PSUM') as ps1:
                pw1_sb = []
                for i in range(CT):
                    t = wp1.tile([128, 2 * CONV_INNER], F32R, tag=f'pw1_{i}')
                    nc.sync.dma_start(t[:], din['w_pw1'].ap()[i * 128:(i + 1) * 128, :])
                    pw1_sb.append(t)
                bpw1_t = _bias_tile(nc, btp, din['b_pw1'], 2 * UT)
                for ut in range(UT):
                    for sc in range(SC):
                        p_a = ps1.tile([128, 512], F32, tag='pp')
                        p_g = ps1.tile([128, 512], F32, tag='pp')
                        for ct in range(CT):
                            nc.tensor.matmul(
                                p_a[:], pw1_sb[ct][:, ut * 128:(ut + 1) * 128],
                                x2_sb[ct][:, sc * 512:(sc + 1) * 512],
                                start=(ct == 0), stop=(ct == CT - 1))
                        for ct in range(CT):
                            nc.tensor.matmul(
                                p_g[:], pw1_sb[ct][:, CONV_INNER + ut * 128:CONV_INNER + (ut + 1) * 128],
                                x2_sb[ct][:, sc * 512:(sc + 1) * 512],
                                start=(ct == 0), stop=(ct == CT - 1))
                        sig = sigp.tile([128, 512], F32, tag='sig')
                        nc.scalar.activation(sig[:], p_g[:], AF.Sigmoid,
                                             bias=bpw1_t[:, UT + ut:UT + ut + 1])
                        nc.vector.scalar_tensor_tensor(
                            u8[:, ut, PAD + sc * 512:PAD + (sc + 1) * 512],
                            p_a[:], bpw1_t[:, ut:ut + 1], sig[:],
                            op0=ALU.add, op1=ALU.mult)

            # depth conv (dense conv1d over seq, K=31) + silu, fp8 DoubleRow
            h_sb = [hid_tile(i) for i in range(UT)]
            with tc.tile_pool(name='wdc', bufs=3) as wdc, \
                 tc.tile_pool(name='ps_dc', bufs=4, space='PSUM') as psd:
                bdc_t = _bias_tile(nc, btp, din['b_dc'], UT)
                for ot in range(UT):
                    ps_c = [psd.tile([128, 512], F32, tag='cv', name=f'cv{_sc}') for _sc in range(SC)]
                    for cp in range(CP):
                        wt = wdc.tile([128, K * 2, 128], F8, tag='dw')
                        nc.sync.dma_start(wt[:], din['w_dc'].ap()[ot, cp])
                        for k in range(K):
                            for sc in range(SC):
                                nc.tensor.matmul(
                                    ps_c[sc][:], wt[:, 2 * k:2 * k + 2, :],
                                    u8[:, 2 * cp:2 * cp + 2,
                                       k + sc * 512:k + sc * 512 + 512],
                                    start=(cp == 0 and k == 0),
                                    stop=(cp == CP - 1 and k == K - 1),
                                    perf_mode=DR)
                    for sc in range(SC):
                        nc.scalar.activation(
                            h_sb[ot][:, sc * 512:(sc + 1) * 512], ps_c[sc][:],
                            AF.Silu, bias=bdc_t[:, ot:ot + 1],
                            scale=float(1.0 / (SW * SU)))

        # pw2 + residual
        x3_sb = []
        with tc.tile_pool(name='wpw2', bufs=1) as wp2, \
             tc.tile_pool(name='ps_pw2', bufs=4, space='PSUM') as ps2:
            pw2_sb = []
            for i in range(UT):
                t = wp2.tile([128, DIM], F32R, tag=f'pw2_{i}')
                nc.sync.dma_start(t[:], din['w_pw2'].ap()[i * 128:(i + 1) * 128, :])
                pw2_sb.append(t)
            bpw2_t = _bias_tile(nc, btp, din['b_pw2'], CT)
            for ot in range(CT):
                t = new_resid(ot)
                for sc in range(SC):
                    p = ps2.tile([128, 512], F32, tag='pp')
                    for ct in range(UT):
                        nc.tensor.matmul(
                            p[:], pw2_sb[ct][:, ot * 128:(ot + 1) * 128],
                            h_sb[ct][:, sc * 512:(sc + 1) * 512],
                            start=(ct == 0), stop=(ct == UT - 1))
                    nc.vector.scalar_tensor_tensor(
                        t[:, sc * 512:(sc + 1) * 512], p[:],
                        bpw2_t[:, ot:ot + 1],
                        x2_sb[ot][:, sc * 512:(sc + 1) * 512],
                        op0=ALU.add, op1=ALU.add)
                x3_sb.append(t)

        # ================= FF2 =================
        x38 = quant_x(x3_sb, 'x38')
        x4_sb = ff_block(x3_sb, x38, din['w_ff2_1'], din['b_ff2_1'],
                         din['w_ff2_2'], 'ff2')

        # ================= final affine + store =================
        with tc.tile_pool(name='fin', bufs=2) as fp:
            fing_t = _bias_tile(nc, btp, din['fin_g'], CT)
            finb_t = _bias_tile(nc, btp, din['fin_b'], CT)
            for ot in range(CT):
                o_t = fp.tile([128, S], F32, tag='out')
                nc.vector.tensor_scalar(
                    o_t[:], x4_sb[ot][:], fing_t[:, ot:ot + 1],
                    finb_t[:, ot:ot + 1], op0=ALU.mult, op1=ALU.add)
                nc.sync.dma_start(out_d.ap()[ot * 128:(ot + 1) * 128, :], o_t[:])


_prog_cache = {}


def _get_program():
    if 'nc' not in _prog_cache:
        _prog_cache['nc'] = build_program()
    return _prog_cache['nc']


def kernel(**inputs):
    inputs = {k: np.asarray(v, dtype=np.float32) for k, v in inputs.items()}
    w = _host_prep(inputs)
    nc = _get_program()
    x = inputs['x'][..., 0]  # [B, DIM, S]
    in_maps = [dict(w, x=np.ascontiguousarray(x[b]), x8=_xq8(x[b]))
               for b in range(N_CORES)]
    res = run_bass_kernel_spmd(nc, in_maps, core_ids=list(range(N_CORES)))
    out = np.stack([res.results[b]['out'] for b in range(N_CORES)])
    return out[..., None].astype(np.float32)

